# revision 14
# baseline (speedup 1.0000x reference)
"""GAT message-passing layer on 8 trn2 NeuronCores.

reference:
    h_prime = einsum('bnf,hfo->bhno', h, w)              [bs,H,n,o]
    src = h_prime @ a_src ; dst = h_prime @ a_dst        [bs,H,n]
    attn = softmax_j(leaky_relu(src_i + dst_j, 0.2))     [bs,H,n,n]
    out = attn @ h_prime + b                             [bs,H,n,o]
    returns (out, attn)

Sharding: data-parallel over bs (16 graphs -> 2 per core).

Device algorithm per (graph, head), per 128-row i-tile:
  - logitsT[j,i] = dst_j + src_i via a rank-2 matmul
    (lhsT rows = (dst, ones), rhs rows = (ones, src)), PSUM [128j, 1024i].
  - leaky relu (ScalarE Prelu alpha=0.2), exp (ScalarE) -> eT in SBUF.
  - out' = sum_j eT.T @ [h_prime | 1]  -> [128i, 65]: col 64 is the softmax
    denominator S_i; out = out'[:, :64] / S_i.
  - TensorE-transpose eT back to natural [i, j] layout; VectorE multiplies by
    1/S_i while copying PSUM->SBUF; DMA to HBM.

src/dst never need h_prime: src = h @ (w @ a_src), dst = h @ (w @ a_dst);
the [32]-vectors w@a are precomputed on host.
"""

import numpy as np

BS, N, F, O, H = 16, 1024, 32, 64, 4
NCORES = 8
G = BS // NCORES  # graphs per core
NT = N // 128  # i-tiles per (g,h)
JB = N // 128  # j-blocks
NEG_SLOPE = 0.2

# leaky-relu engine per tile index: True -> ScalarE Prelu, False -> VectorE
# scalar_tensor_tensor max(x, 0.2x). Mixed to balance engine load.
SE_LEAKY_FRAC = 1.0  # start all on ScalarE; tune after profiling

_CACHE = {}


def _build_program(reps=1):
    """reps>1 wraps the main loop in an on-device For loop (benchmarking
    only: every iteration recomputes and rewrites the same outputs)."""
    import concourse.mybir as mybir
    import concourse.tile as tile
    from concourse import bacc
    from concourse.masks import make_identity

    f32 = mybir.dt.float32
    nc = bacc.Bacc(None, target_bir_lowering=False)

    FA = F + 1  # h augmented with a ones column on host
    hx = nc.dram_tensor("hx", [G, N, FA], f32, kind="ExternalInput")
    Wt = nc.dram_tensor("Wt", [F, H * O], f32, kind="ExternalInput")
    # per-head mixing matrix: R_h = MT_h.T @ hT_aug gives rows 0..31 =
    # wa_dst[f] (constant over i) and row 32 = src_i
    MTt = nc.dram_tensor("MTt", [FA, H * FA], f32, kind="ExternalInput")
    outp = nc.dram_tensor("outp", [G, H, N, O], f32, kind="ExternalOutput")
    attnp = nc.dram_tensor("attnp", [G, H, N, N], f32, kind="ExternalOutput")

    with tile.TileContext(nc) as tc:
        import contextlib

        with contextlib.ExitStack() as ctx:
            singles = ctx.enter_context(tc.tile_pool(name="singles", bufs=1))
            work = ctx.enter_context(tc.tile_pool(name="work", bufs=2))
            attn_sb_pool = ctx.enter_context(tc.tile_pool(name="attnsb", bufs=3))
            small = ctx.enter_context(tc.tile_pool(name="small", bufs=3))
            psumA = ctx.enter_context(tc.tile_pool(name="psumA", bufs=2, space="PSUM"))
            psumB = ctx.enter_context(tc.tile_pool(name="psumB", bufs=2, space="PSUM"))

            ident = singles.tile([128, 128], f32, tag="ident")
            make_identity(nc, ident)

            W_sb = singles.tile([F, H * O], f32, tag="W_sb")
            nc.sync.dma_start(W_sb, Wt[:, :])
            MT_sb = singles.tile([FA, H * FA], f32, tag="MT_sb")
            nc.sync.dma_start(MT_sb, MTt[:, :])

            hT = []  # per graph: [33, 1024] augmented h transposed
            R = {}  # (g, head) -> [33, 1024]: rows 0..31 wa_dst bcast, row 32 src
            hp_aug = {}  # (g, head) -> [128, JB, O+1]

            for g in range(G):
                h_nat = singles.tile([128, NT, FA], f32, tag=f"h_nat{g}")
                nc.sync.dma_start(
                    h_nat, hx[g].rearrange("(t p) f -> p t f", p=128)
                )
                hT_ps = psumA.tile([FA, N], f32, tag="big")
                for t in range(NT):
                    nc.tensor.transpose(
                        hT_ps[:, t * 128 : (t + 1) * 128], h_nat[:, t, :], ident
                    )
                hT_g = singles.tile([FA, N], f32, tag=f"hT{g}")
                nc.scalar.copy(hT_g, hT_ps)
                hT.append(hT_g)

                for hh in range(H):
                    R_ps = psumA.tile([FA, N], f32, tag="big")
                    MT_h = MT_sb[:, hh * FA : (hh + 1) * FA]
                    nc.tensor.matmul(R_ps[:, 0:512], MT_h, hT_g[:, 0:512])
                    nc.tensor.matmul(R_ps[:, 512:1024], MT_h, hT_g[:, 512:1024])
                    R_t = singles.tile([FA, N], f32, tag=f"R{g}_{hh}")
                    nc.scalar.copy(R_t, R_ps)
                    R[(g, hh)] = R_t

                    hp_ps = psumB.tile([128, JB * O], f32, tag="attnT")
                    for jb in range(JB):
                        nc.tensor.matmul(
                            hp_ps[:, jb * O : (jb + 1) * O],
                            hT_g[0:F, jb * 128 : (jb + 1) * 128],
                            W_sb[:, hh * O : (hh + 1) * O],
                        )
                    hp = singles.tile([128, JB, O + 1], f32, tag=f"hp{g}_{hh}")
                    nc.scalar.copy(
                        hp[:, :, 0:O], hp_ps.rearrange("p (t o) -> p t o", o=O)
                    )
                    nc.vector.memset(hp[:, :, O : O + 1], 1.0)
                    hp_aug[(g, hh)] = hp

            def emit_main():
                tile_idx = 0
                for g in range(G):
                    for hh in range(H):
                        emit_bh(g, hh, tile_idx)
                        tile_idx += NT

            def emit_bh(g, hh, tile_idx):
                    Rp = R[(g, hh)]
                    hp = hp_aug[(g, hh)]
                    for it in range(NT):
                        isl = slice(it * 128, (it + 1) * 128)
                        p1 = psumA.tile([128, N], f32, tag="big")
                        for jb in range(JB):
                            nc.tensor.matmul(
                                p1[:, jb * 128 : (jb + 1) * 128],
                                hT[g][:, jb * 128 : (jb + 1) * 128],
                                Rp[:, isl],
                            )
                        # leaky relu
                        L = work.tile([128, N], f32, tag="L")
                        use_se = (tile_idx * SE_LEAKY_FRAC) % 1.0 < SE_LEAKY_FRAC
                        if SE_LEAKY_FRAC >= 1.0 or use_se:
                            nc.scalar.activation(
                                L, p1, mybir.ActivationFunctionType.Prelu,
                                alpha=NEG_SLOPE,
                            )
                        else:
                            nc.vector.scalar_tensor_tensor(
                                L, p1, NEG_SLOPE, p1,
                                mybir.AluOpType.mult, mybir.AluOpType.max,
                            )
                        eT = work.tile([128, N], f32, tag="eT")
                        nc.scalar.activation(
                            eT, L, mybir.ActivationFunctionType.Exp
                        )
                        # out' = sum_j eT_jb.T @ [hp | 1]  -> [128, O+1] in p1
                        for jb in range(JB):
                            nc.tensor.matmul(
                                p1[:, 0 : O + 1],
                                eT[:, jb * 128 : (jb + 1) * 128],
                                hp[:, jb, :],
                                start=(jb == 0),
                                stop=(jb == JB - 1),
                            )
                        recipS = small.tile([128, 1], f32, tag="recipS")
                        nc.vector.reciprocal(recipS, p1[:, O : O + 1])
                        # transpose eT back to natural layout
                        aT = psumB.tile([128, N], f32, tag="attnT")
                        for jb in range(JB):
                            nc.tensor.transpose(
                                aT[:, jb * 128 : (jb + 1) * 128],
                                eT[:, jb * 128 : (jb + 1) * 128],
                                ident,
                            )
                        attn_sb = attn_sb_pool.tile([128, N], f32, tag="attn_sb")
                        nc.vector.tensor_scalar(
                            attn_sb, aT, recipS, None, mybir.AluOpType.mult
                        )
                        nc.sync.dma_start(attnp[g, hh, isl, :], attn_sb)
                        out_sb = small.tile([128, O], f32, tag="out_sb")
                        nc.vector.tensor_scalar(
                            out_sb, p1[:, 0:O], recipS, None, mybir.AluOpType.mult
                        )
                        nc.sync.dma_start(outp[g, hh, isl, :], out_sb)
                        tile_idx += 1

            if reps == 1:
                emit_main()
            else:
                with tc.For_i(0, reps, 1, hint_engines=(mybir.EngineType.PE,)):
                    emit_main()

    nc.compile()
    return nc


def _get_program():
    if "nc" not in _CACHE:
        _CACHE["nc"] = _build_program()
    return _CACHE["nc"]


def run(h, w, a_src, a_dst, b, trace=False):
    from concourse.bass_utils import run_bass_kernel_spmd

    h = np.ascontiguousarray(np.asarray(h, dtype=np.float32))
    w = np.asarray(w, dtype=np.float32)
    a_src = np.asarray(a_src, dtype=np.float32)
    a_dst = np.asarray(a_dst, dtype=np.float32)
    b = np.asarray(b, dtype=np.float32)

    # host precompute: fold w @ a into per-head [F] vectors
    wa_src = np.einsum("hfo,ho->hf", w, a_src[..., 0])  # [H, F]
    wa_dst = np.einsum("hfo,ho->hf", w, a_dst[..., 0])  # [H, F]
    Wf = np.transpose(w, (1, 0, 2)).reshape(F, H * O).copy()  # [F, H*O]

    FA = F + 1
    h_aug = np.concatenate(
        [h, np.ones((BS, N, 1), np.float32)], axis=2
    )  # [BS, N, 33]
    # MT[h][g, f]: R_h[f, i] = sum_g MT[h][g, f] * hT_aug[g, i]
    #   f < F : wa_dst[h, f]   (g = F picks the ones row)
    #   f = F : src_i          (g < F rows carry wa_src)
    MT = np.zeros((H, FA, FA), np.float32)
    for hh in range(H):
        MT[hh, F, 0:F] = wa_dst[hh]
        MT[hh, 0:F, F] = wa_src[hh]
    MTf = np.transpose(MT, (1, 0, 2)).reshape(FA, H * FA).copy()

    nc = _get_program()
    in_maps = [
        {
            "hx": np.ascontiguousarray(h_aug[c * G : (c + 1) * G]),
            "Wt": Wf,
            "MTt": MTf,
        }
        for c in range(NCORES)
    ]
    res = run_bass_kernel_spmd(
        nc, in_maps, core_ids=list(range(NCORES)), trace=trace
    )
    out = np.concatenate([r["outp"] for r in res.results], axis=0)
    attn = np.concatenate([r["attnp"] for r in res.results], axis=0)
    out = out + b[None, None, None, :]
    return (out, attn), res


def kernel(h, w, a_src, a_dst, b):
    (out, attn), _ = run(h, w, a_src, a_dst, b)
    return out, attn
